# revision 34
# baseline (speedup 1.0000x reference)
"""GATNet (2x GATConv + MLP head + log_softmax) on 8 Trainium2 NeuronCores.

Strategy (dst-partitioned message passing, v3):
  - Host assigns destination nodes to 8 devices x SPD slots (32 nodes/slot),
    balancing in-edge counts so every slot has exactly TPS 128-edge tiles.
    Every device runs an identical program; per-device data differ.
  - Layer tables T = [h(64) | a_s(4) | pad] are single 256B bf16 rows, one per
    node, [ng, 128].  int16 gather indices only reach 32767, so each slot's
    edges are sorted low-half sources first and every chunk runs TWO
    dma_gathers: A over tiles [0, nA) against table rows [0, ng/2), B over
    tiles [nB0, tps) against rows [ng/2, ng).  Only ~(nA+tps-nB0)/tps of the
    pair-gather bytes move, which is what the gather costs on the Pool queue.
  - Per chunk (4 slots = 128 dst nodes) attention is built edge-wise (bf16),
    low/high exp weights are region-masked, and messages aggregate into one
    68-wide PSUM accumulator at 4 partition offsets via one-hot matmuls.
  - Layer-2 table rows are produced inside layer 1's epilogue (each device
    computes rows only for its own nodes = device-major global numbering) and
    shared with a single AllGather between the layers.
  - a_e for both layers is computed once in layer 1 and parked in SBUF; a_d
    tables also stay resident in SBUF.  log_softmax's Ln runs once at the end
    (avoids per-chunk activation-table reloads); output leaves in one DMA.
"""

import numpy as np

# model constants (fixed by the problem)
IN = 128
HID = 16
OUT = 40
H = 4
ED = 16
HC = 64  # HID * H
NEG = 0.2
EPS = 1e-16

C = 8          # NeuronCores
NSLOT = 32     # nodes per slot (= one-hot width, PSUM col-block)


def _bf16():
    import concourse.mybir as mybir
    return mybir.dt.np(mybir.dt.bfloat16)


# ----------------------------------------------------------------------------
# host-side plan: balance nodes into (device, slot) bins, lay out edge shards
# ----------------------------------------------------------------------------

def _build_plan(src, dst, n_nodes):
    """Returns a dict with the full sharding plan. src/dst include self-loops."""
    import heapq

    deg = np.bincount(dst, minlength=n_nodes).astype(np.int64)
    e_tot = src.shape[0]

    def try_pack(nbins, cap_e, lo_deg=None, cap_lo=None, cap_hi=None):
        # LPT: heaviest nodes first into least-loaded feasible bin; optional
        # second dimension caps the low-half / high-half in-edge loads.
        order = np.argsort(-deg, kind="stable")
        loads = [(0, b) for b in range(nbins)]
        heapq.heapify(loads)
        bin_of_t = np.empty(n_nodes, np.int64)
        bin_cnt = np.zeros(nbins, np.int64)
        bin_load = np.zeros(nbins, np.int64)
        bin_lo = np.zeros(nbins, np.int64)
        for nd in order:
            d = int(deg[nd])
            lo = int(lo_deg[nd]) if lo_deg is not None else 0
            hi = d - lo
            spill = []
            placed = False
            while loads:
                l, b = heapq.heappop(loads)
                ok = bin_cnt[b] < NSLOT and bin_load[b] + d <= cap_e
                if ok and lo_deg is not None:
                    ok = (bin_lo[b] + lo <= cap_lo
                          and (bin_load[b] + d) - (bin_lo[b] + lo) <= cap_hi)
                if ok:
                    bin_of_t[nd] = b
                    bin_cnt[b] += 1
                    bin_load[b] += d
                    bin_lo[b] += lo
                    heapq.heappush(loads, (bin_load[b], b))
                    placed = True
                    break
                elif bin_cnt[b] < NSLOT:
                    spill.append((l, b))
                # full bins are dropped
            for it in spill:
                heapq.heappush(loads, it)
            if not placed:
                return None
        return bin_of_t

    # search (slots-per-device, even tiles-per-slot) minimizing total tiles
    spd_min = 4 * int(np.ceil(n_nodes / (C * NSLOT * 4)))  # node-capacity floor
    best = None  # (tq, spd, tps, bin_of)
    for spd_try in range(spd_min, spd_min + 65, 4):
        nbins = C * spd_try
        tps_lo = int(np.ceil(e_tot / nbins / 128.0))
        tps_lo += tps_lo % 2  # ch = 4*tps must be a multiple of 8
        for tps_try in (tps_lo, tps_lo + 2):
            if best is not None and spd_try * tps_try >= best[0]:
                continue
            got = try_pack(nbins, tps_try * 128)
            if got is not None:
                best = (spd_try * tps_try, spd_try, tps_try, got)
                break
        if best is not None and (spd_try + 4) * 2 >= best[0]:
            break
    assert best is not None, "balancer failed"
    _, spd, tps, bin_of = best

    nbins = C * spd
    npd = spd * NSLOT
    ng = C * npd

    def finish(bin_of):
        # position of each node within its bin; device-major global numbering
        pos_of = np.zeros(n_nodes, np.int64)
        fill = np.zeros(nbins, np.int64)
        for nd in range(n_nodes):
            b = bin_of[nd]
            pos_of[nd] = fill[b]
            fill[b] += 1
        node2g = ((bin_of // spd) * npd + (bin_of % spd) * NSLOT + pos_of
                  ).astype(np.int64)
        return pos_of, node2g

    pos_of, node2g = finish(bin_of)

    # table half boundary: low rows [0, X), high rows [X, ng).  Both gather
    # index spaces must fit int16.
    X = int(round(0.4435 * ng / 128.0)) * 128
    X = min(X, 32768)
    assert ng - X <= 32768

    # re-pack with per-bin low/high in-edge caps so the two gather spans are
    # as narrow as possible (lo <= 500 -> nA=4; hi <= 615 -> nB0>=3 for full
    # bins).  Low-degrees come from the first pack's numbering; the re-pack
    # perturbs halves only slightly, so re-measure and take actual spans.
    lo_src = (node2g[src] < X)
    lo_deg_nd = np.bincount(dst[lo_src], minlength=n_nodes).astype(np.int64)
    repack = None
    for cap_lo, cap_hi in ((500, 600), (500, 615), (511, 630)):
        repack = try_pack(nbins, tps * 128, lo_deg=lo_deg_nd,
                          cap_lo=cap_lo, cap_hi=cap_hi)
        if repack is not None:
            break
    if repack is not None:
        bin_of = repack
        pos_of, node2g = finish(bin_of)

    # edges per destination bin, LOW-half sources first, then high, then pad
    ebin = bin_of[dst]
    half = (node2g[src] >= X).astype(np.int64)
    order = np.lexsort((half, ebin))          # by bin, low-src first (stable)
    counts = np.bincount(ebin, minlength=nbins)
    lowcnt = np.bincount(ebin[half == 0], minlength=nbins)
    cap = tps * 128
    assert counts.max() <= cap
    starts = np.zeros(nbins + 1, np.int64)
    np.cumsum(counts, out=starts[1:])
    rank = np.arange(e_tot, dtype=np.int64) - starts[ebin[order]]
    canvas = np.full((nbins, cap), -1, np.int64)       # edge id or -1 pad
    canvas[ebin[order], rank] = order

    # static gather spans: A covers tiles [0, nA), B covers [nB0, tps)
    nA = int(np.ceil(lowcnt.max() / 128.0))
    nB0 = int(lowcnt.min() // 128)
    assert 0 <= nB0 <= nA <= tps

    return dict(
        spd=spd, tps=tps, npd=npd, ng=ng, nbins=nbins, qpd=spd // 4,
        nA=nA, nB0=nB0, X=X, bin_of=bin_of, pos_of=pos_of, node2g=node2g,
        canvas=canvas,
    )


def _host_arrays(plan, x, src, dst, edge_attr, mean_attr, n_nodes, Ve):
    """Per-core input arrays."""
    bf16 = _bf16()
    spd, tps, npd, ng = plan["spd"], plan["tps"], plan["npd"], plan["ng"]
    nA, nB0, X = plan["nA"], plan["nB0"], plan["X"]
    nBn = tps - nB0
    node2g, pos_of, canvas = plan["node2g"], plan["pos_of"], plan["canvas"]
    tq = spd * tps                       # 128-edge tiles per device
    ch = 4 * tps
    nq = tq // ch                        # chunks
    e0 = edge_attr.shape[0]

    def wrap16(a):  # [n] i16 -> [128, n//16] gather-index wrapping
        return np.tile(a.reshape(-1, 16).T, (8, 1))

    # permuted node features, transposed: xT [IN, ng] (bf16)
    xg = np.zeros((ng, IN), np.float32)
    xg[node2g] = np.asarray(x, np.float32)
    xT = np.ascontiguousarray(xg.T.astype(bf16))

    per_core = []
    for d in range(C):
        cv = canvas[d * spd:(d + 1) * spd].reshape(tq, 128)  # [tile, lane]
        valid = cv >= 0
        eid = np.where(valid, cv, 0)
        srcg = np.where(valid, node2g[src[eid]], 0)          # [tq, 128]
        hi = (srcg >= X)
        idxA = np.where(hi, 0, srcg).astype(np.int16)
        idxB = np.where(hi, srcg - X, 0).astype(np.int16)
        m4 = np.repeat(hi.astype(bf16).T[:, :, None], 4, axis=2
                       ).reshape(128, tq * 4).view(np.int16)  # [128, tq*4]
        drel = np.where(valid, pos_of[dst[eid]], -1)          # [tq, 128]
        # host-built one-hot S: [p, chunk, (b j s)] (bf16 bits)
        S_all = (drel[:, :, None] == np.arange(NSLOT)).astype(bf16)
        Sfull = (S_all.reshape(nq, 4, tps, 128, NSLOT)
                 .transpose(3, 0, 2, 1, 4)
                 .reshape(128, nq, tps * 4 * NSLOT)).view(np.int16)

        # host-computed a_e for both layers: [128, tq, 8] bf16 bits
        ea = np.zeros((tq, 128, ED), np.float32)
        sel = valid & (eid < e0)
        ea[sel] = edge_attr[eid[sel]]
        loop_sel = valid & (eid >= e0)
        ea[loop_sel] = mean_attr
        ae8 = (ea.reshape(-1, ED) @ Ve).reshape(tq, 128, 8)
        ae8 = np.ascontiguousarray(ae8.transpose(1, 0, 2)).astype(bf16)

        # per-chunk comb block: [idxA | idxB | m4 | ae1 | ae2 | S] int16
        cwA, cwB = 32 * nA, 32 * nBn
        cw = cwA + cwB + 12 * ch + NSLOT * ch
        comb = np.empty((128, nq * cw), np.int16)
        t4 = np.arange(tq).reshape(nq, 4, tps)                # chunk, j, tt
        ae1b = ae8[:, :, 0:4].reshape(128, tq * 4).view(np.int16)
        ae2b = ae8[:, :, 4:8].reshape(128, tq * 4).view(np.int16)
        for q in range(nq):
            blk = comb[:, q * cw:(q + 1) * cw]
            tA = t4[q, :, 0:nA].reshape(-1)                   # u = j*nA+tt
            tB = t4[q, :, nB0:tps].reshape(-1)
            o = 0
            blk[:, o:o + cwA] = wrap16(idxA[tA].reshape(-1)); o += cwA
            blk[:, o:o + cwB] = wrap16(idxB[tB].reshape(-1)); o += cwB
            blk[:, o:o + 4 * ch] = m4[:, q * 4 * ch:(q + 1) * 4 * ch]; o += 4 * ch
            blk[:, o:o + 4 * ch] = ae1b[:, q * 4 * ch:(q + 1) * 4 * ch]; o += 4 * ch
            blk[:, o:o + 4 * ch] = ae2b[:, q * 4 * ch:(q + 1) * 4 * ch]; o += 4 * ch
            blk[:, o:] = Sfull[:, q, :]
        per_core.append(dict(
            comb=np.ascontiguousarray(comb),
            xTloc=np.ascontiguousarray(xT[:, d * npd:(d + 1) * npd]),
        ))
    return per_core, xT, tq


def _fold_weights(W1, att_s1, att_d1, We1, att_e1, b1,
                  W2, att_s2, att_d2, We2, att_e2, b2,
                  lw1, lb1, lw2, lb2):
    bf16 = _bf16()

    def head_fold(att):  # [H, HID] -> [HC, H] block diag columns
        A = np.zeros((HC, H), np.float32)
        for h in range(H):
            A[h * HID:(h + 1) * HID, h] = att[h]
        return A

    W1aug = np.concatenate([W1, W1 @ head_fold(att_s1), W1 @ head_fold(att_d1)], 1)
    W2aug = np.concatenate([W2, W2 @ head_fold(att_s2), W2 @ head_fold(att_d2)], 1)
    Ve = np.zeros((ED, 8), np.float32)
    for h in range(H):
        Ve[:, h] = We1[:, h * HID:(h + 1) * HID] @ att_e1[h]
        Ve[:, 4 + h] = We2[:, h * HID:(h + 1) * HID] @ att_e2[h]
    LW = (lw1 @ lw2).astype(np.float32)
    lb2p = (lb1 @ lw2 + lb2).astype(np.float32)
    return (W1aug.astype(bf16), W2aug.astype(bf16), Ve,
            LW.astype(bf16), lb2p, b1.astype(np.float32), b2.astype(np.float32))


# ----------------------------------------------------------------------------
# the bass program (identical for all cores)
# ----------------------------------------------------------------------------

def _build_nc(ng, npd, spd, tps, tq, nA, nB0, X):
    import concourse.bass as bass
    import concourse.mybir as mybir
    import concourse.tile as tile
    from concourse import bacc
    from contextlib import ExitStack

    F32 = mybir.dt.float32
    BF16 = mybir.dt.bfloat16
    I16 = mybir.dt.int16
    ALU = mybir.AluOpType
    ACT = mybir.ActivationFunctionType

    ch = 4 * tps          # tiles per chunk (one quad = 4 slots)
    qpd = spd // 4        # chunks per device per layer
    nt = ng // 128        # node tiles (table build)
    jpd = npd // 128      # local 128-node groups (== qpd)
    ngr = ch // 8         # eaTg groups per chunk
    nBn = tps - nB0
    ov = nA - nB0         # mixed tiles per slot
    uA, uB = 4 * nA, 4 * nBn
    cwA, cwB = 32 * nA, 32 * nBn
    cw = cwA + cwB + 12 * ch + NSLOT * ch

    nc = bacc.Bacc(None, target_bir_lowering=False)

    # kernel IO
    t_xT = nc.dram_tensor("xT", [128, ng], BF16, kind="ExternalInput")
    t_xTl = nc.dram_tensor("xTloc", [128, npd], BF16, kind="ExternalInput")
    t_comb = nc.dram_tensor("comb", [128, (tq // ch) * cw], I16, kind="ExternalInput")
    t_W1 = nc.dram_tensor("W1aug", [128, 72], BF16, kind="ExternalInput")
    t_W2 = nc.dram_tensor("W2aug", [64, 72], BF16, kind="ExternalInput")
    t_LW = nc.dram_tensor("LW", [64, OUT], BF16, kind="ExternalInput")
    t_cst = nc.dram_tensor("cst", [1, 256], F32, kind="ExternalInput")
    # cst row: [b1(64) | b2(64) | lb2p(40) | iota32(32) | pad]
    t_mask = nc.dram_tensor("bdmask", [128, 16], BF16, kind="ExternalInput")
    t_I = nc.dram_tensor("ident", [128, 128], F32, kind="ExternalInput")
    t_out = nc.dram_tensor("out", [128, qpd * OUT], F32, kind="ExternalOutput")

    # node tables: tight 136B rows for builds/collective; padded 256B rows
    # (DRAM->DRAM expanded) for the 256B-granularity gathers
    I32 = mybir.dt.int32
    d_T1t = nc.dram_tensor("T1t", [ng, 68], BF16)
    d_T1 = nc.dram_tensor("T1", [ng, 64], I32)
    d_T2loc = nc.dram_tensor("T2loc", [npd, 68], BF16)
    d_T2t = nc.dram_tensor("T2t", [ng, 68], BF16, addr_space="Shared")
    d_T2 = nc.dram_tensor("T2", [ng, 64], I32)

    with tile.TileContext(nc) as tc, ExitStack() as top:
        cp = top.enter_context(tc.tile_pool(name="consts", bufs=1))
        pers = top.enter_context(tc.tile_pool(name="persist", bufs=1))

        W1sb = cp.tile([128, 72], BF16)
        W2sb = cp.tile([64, 72], BF16)
        LWsb = cp.tile([64, OUT], BF16)
        Isb = cp.tile([128, 128], F32)
        maskb = cp.tile([128, 16], BF16)
        b1bc = cp.tile([128, 64], F32)
        b2bc = cp.tile([128, 64], F32)
        lbbc = cp.tile([128, OUT], F32)
        Ib16 = cp.tile([128, 128], BF16)
        nc.sync.dma_start(W1sb[:], t_W1[:, :])
        nc.sync.dma_start(W2sb[:], t_W2[:, :])
        nc.sync.dma_start(LWsb[:], t_LW[:, :])
        nc.sync.dma_start(Isb[:], t_I[:, :])
        nc.sync.dma_start(maskb[:], t_mask[:, :])
        nc.sync.dma_start(b1bc[:], t_cst[:, 0:64].partition_broadcast(128))
        nc.sync.dma_start(b2bc[:], t_cst[:, 64:128].partition_broadcast(128))
        nc.sync.dma_start(lbbc[:], t_cst[:, 128:128 + OUT].partition_broadcast(128))
        nc.vector.tensor_copy(out=Ib16[:], in_=Isb[:])

        # persistent SBUF state
        ad1sb = pers.tile([128, jpd, 4], BF16)      # layer-1 a_d per local node
        ad2sb = pers.tile([128, jpd, 4], BF16)      # layer-2 a_d per local node
        zall = pers.tile([128, qpd, OUT], F32)      # head logits (shifted)
        smsb = pers.tile([128, qpd], F32)           # softmax sums

        # ---------------- phase A1: T1 = [x@W1 | a_s1]; local a_d1 ----------
        with ExitStack() as ph:
            ap = ph.enter_context(tc.tile_pool(name="pa_sb", bufs=3))
            app = ph.enter_context(tc.tile_pool(name="pa_ps", bufs=2, space="PSUM"))
            for it, i0 in enumerate(range(0, nt, 8)):
                bs = min(8, nt - i0)
                xt = ap.tile([128, 8 * 128], BF16, tag="xt")
                ldeng = nc.sync if it % 2 == 0 else nc.scalar
                ldeng.dma_start(xt[:, 0:128 * bs], t_xT[:, 128 * i0:128 * (i0 + bs)])
                ps0 = app.tile([128, 4, 72], F32, tag="ps0")
                ps1 = app.tile([128, 4, 72], F32, tag="ps1")
                for c in range(bs):
                    pst = ps0 if c < 4 else ps1
                    nc.tensor.matmul(pst[:, c % 4, :], xt[:, 128 * c:128 * (c + 1)],
                                     W1sb[:], start=True, stop=True)
                hsb = ap.tile([128, 8, 68], BF16, tag="hsb")
                nc.vector.tensor_copy(out=hsb[:, 0:4, :], in_=ps0[:, :, 0:68])
                if bs > 4:
                    nc.vector.tensor_copy(out=hsb[:, 4:bs, :],
                                          in_=ps1[:, 0:bs - 4, 0:68])
                nc.gpsimd.dma_start(
                    d_T1t.ap()[128 * i0:128 * (i0 + bs), :].rearrange(
                        "(c r) d -> r c d", c=bs),
                    hsb[:, 0:bs, :])
            for jj0 in range(0, jpd, 8):
                bs = min(8, jpd - jj0)
                xt = ap.tile([128, 8 * 128], BF16, tag="xt")
                nc.sync.dma_start(xt[:, 0:128 * bs], t_xTl[:, 128 * jj0:128 * (jj0 + bs)])
                psa = app.tile([128, 32], F32, tag="psa")
                for c in range(bs):
                    nc.tensor.matmul(psa[:, 4 * c:4 * (c + 1)],
                                     xt[:, 128 * c:128 * (c + 1)],
                                     W1sb[:, 68:72], start=True, stop=True)
                nc.vector.tensor_copy(
                    out=ad1sb[:, jj0:jj0 + bs, :],
                    in_=psa[:, 0:4 * bs].rearrange("p (c v) -> p c v", v=4))
            # expand tight rows into the 256B-granularity gather table
            nc.sync.dma_start(d_T1.ap().bitcast(BF16)[:, 0:68], d_T1t.ap()[:, :])

        # ---------------- edge phase (shared for both layers) ----------------
        def edge_layer(layer, tbl, adsb):
            with ExitStack() as ph:
                ip = ph.enter_context(tc.tile_pool(name=f"l{layer}_idx", bufs=3))
                gp = ph.enter_context(tc.tile_pool(name=f"l{layer}_g", bufs=3))
                sp = ph.enter_context(tc.tile_pool(name=f"l{layer}_s", bufs=2))
                mp = ph.enter_context(tc.tile_pool(name=f"l{layer}_m", bufs=2))
                ep = ph.enter_context(tc.tile_pool(name=f"l{layer}_e", bufs=2))
                pp = ph.enter_context(tc.tile_pool(name=f"l{layer}_ps", bufs=2, space="PSUM"))
                p1 = ph.enter_context(tc.tile_pool(name=f"l{layer}_p1", bufs=2, space="PSUM"))

                tlo = tbl.ap()[0:X, :]
                thi = tbl.ap()[X:ng, :]

                for q in range(qpd):
                    c0 = ch * q
                    comb = ip.tile([128, cw], I16, tag="comb")
                    nc.sync.dma_start(comb[:], t_comb[:, cw * q:cw * (q + 1)])
                    idxAv = comb[:, 0:cwA]
                    idxBv = comb[:, cwA:cwA + cwB]
                    m4 = comb[:, cwA + cwB:cwA + cwB + 4 * ch].bitcast(BF16)
                    m4v = m4.rearrange("p (j b v) -> p j b v", j=4, v=4)
                    aeo = cwA + cwB + 4 * ch + (0 if layer == 1 else 4 * ch)
                    aev = (comb[:, aeo:aeo + 4 * ch].bitcast(BF16)
                           .rearrange("p (t v) -> p t v", v=4))
                    Sv = (comb[:, cwA + cwB + 12 * ch:cw].bitcast(BF16)
                          .rearrange("p (b j s) -> p b j s", j=4, s=NSLOT))

                    gA8 = gp.tile([128, uA, 64], I32, tag="gA")
                    nc.gpsimd.dma_gather(
                        out_ap=gA8[:], in_ap=tlo, idxs_ap=idxAv,
                        num_idxs=uA * 128, num_idxs_reg=uA * 128, elem_size=64,
                        single_packet=False)
                    gB8 = gp.tile([128, uB, 64], I32, tag="gB")
                    nc.gpsimd.dma_gather(
                        out_ap=gB8[:], in_ap=thi, idxs_ap=idxBv,
                        num_idxs=uB * 128, num_idxs_reg=uB * 128, elem_size=64,
                        single_packet=False)
                    gA = gA8[:].bitcast(BF16)
                    gB = gB8[:].bitcast(BF16)
                    gAv = gA.rearrange("p (j u) d -> p j u d", j=4)
                    gBv = gB.rearrange("p (j u) d -> p j u d", j=4)

                    # --- a_d expansion: S^T via PE, block-diag a_d matmul
                    bd = ip.tile([128, 16], BF16, tag="bd")
                    nc.vector.tensor_tensor(
                        out=bd[:],
                        in0=adsb[:, q, :].unsqueeze(1).to_broadcast([128, 4, 4]),
                        in1=maskb[:].rearrange("p (j v) -> p j v", v=4),
                        op=ALU.mult)
                    stp = p1.tile([128, tps * 128], BF16, tag="stp")
                    for b in range(tps):
                        nc.tensor.transpose(
                            stp[:, 128 * b:128 * (b + 1)],
                            Sv[:, b, :, :].rearrange("p a w -> p (a w)"), Ib16[:])
                    sts = sp.tile([128, tps * 128], BF16, tag="sts")
                    nc.scalar.activation(sts[:], stp[:], ACT.Copy)
                    scr = p1.tile([128, 328], F32, tag="scr")
                    alad = scr[:, 0:tps * 16]
                    for b in range(tps):
                        nc.tensor.matmul(alad[:, 16 * b:16 * (b + 1)],
                                         sts[:, 128 * b:128 * (b + 1)],
                                         bd[:], start=True, stop=True)
                    aladb = ep.tile([128, tps * 16], BF16, tag="aladb")
                    nc.scalar.activation(aladb[:], alad[:], ACT.Copy)

                    # --- alpha = a_s[src](half-sel) + a_d[dst] + a_e
                    al = mp.tile([128, ch, 4], BF16, tag="al")
                    tp = scr[:, 128:256]
                    alv = al[:].rearrange("p (j b) v -> p j b v", j=4)
                    if nB0 > 0:
                        nc.vector.tensor_copy(out=alv[:, :, 0:nB0, :],
                                              in_=gAv[:, :, 0:nB0, 64:68])
                    if nA < tps:
                        nc.vector.tensor_copy(out=alv[:, :, nA:tps, :],
                                              in_=gBv[:, :, ov:nBn, 64:68])
                    if ov > 0:
                        tmp = mp.tile([128, 4, ov, 4], BF16, tag="tmp")
                        nc.vector.tensor_tensor(
                            out=tmp[:], in0=gBv[:, :, 0:ov, 64:68],
                            in1=gAv[:, :, nB0:nA, 64:68], op=ALU.subtract)
                        nc.vector.tensor_tensor(
                            out=tmp[:], in0=tmp[:],
                            in1=m4v[:, :, nB0:nA, :], op=ALU.mult)
                        nc.vector.tensor_tensor(
                            out=alv[:, :, nB0:nA, :], in0=gAv[:, :, nB0:nA, 64:68],
                            in1=tmp[:], op=ALU.add)
                    nc.vector.tensor_tensor(out=al[:], in0=al[:], in1=aev,
                                            op=ALU.add)
                    # += a_d (tile (j, b) lives at aladb[:, 16b + 4j : +4])
                    nc.vector.tensor_tensor(
                        out=al[:], in0=al[:],
                        in1=aladb[:].rearrange("p (b j v) -> p j b v", j=4, v=4),
                        op=ALU.add)
                    # leaky relu + exp (bf16)
                    lk = mp.tile([128, ch, 4], BF16, tag="lk")
                    nc.vector.tensor_scalar_mul(lk[:], al[:], NEG)
                    nc.vector.tensor_tensor(out=lk[:], in0=al[:], in1=lk[:], op=ALU.max)
                    exb = mp.tile([128, ch, 4], BF16, tag="exb")
                    nc.scalar.activation(exb[:], lk[:], ACT.Exp)
                    exbv = exb[:].rearrange("p (j b) v -> p j b v", j=4)

                    # region-masked exp weights: exLo (A tiles), exHi (B tiles)
                    exHi = mp.tile([128, 4, nBn, 4], BF16, tag="exHi")
                    if nA < tps:
                        nc.vector.tensor_copy(out=exHi[:, :, ov:nBn, :],
                                              in_=exbv[:, :, nA:tps, :])
                    if ov > 0:
                        nc.vector.tensor_tensor(
                            out=exHi[:, :, 0:ov, :], in0=exbv[:, :, nB0:nA, :],
                            in1=m4v[:, :, nB0:nA, :], op=ALU.mult)
                    exLo = mp.tile([128, 4, nA, 4], BF16, tag="exLo")
                    if nB0 > 0:
                        nc.vector.tensor_copy(out=exLo[:, :, 0:nB0, :],
                                              in_=exbv[:, :, 0:nB0, :])
                    if ov > 0:
                        nc.vector.tensor_tensor(
                            out=exLo[:, :, nB0:nA, :], in0=exbv[:, :, nB0:nA, :],
                            in1=exHi[:, :, 0:ov, :], op=ALU.subtract)
                    exLo2 = mp.tile([128, uA, 4, 2], BF16, tag="exLo2")
                    nc.scalar.activation(
                        exLo2[:],
                        exLo[:].rearrange("p j u v -> p (j u) v")
                        .unsqueeze(3).to_broadcast([128, uA, 4, 2]), ACT.Copy)
                    exHi2 = mp.tile([128, uB, 4, 2], BF16, tag="exHi2")
                    nc.scalar.activation(
                        exHi2[:],
                        exHi[:].rearrange("p j u v -> p (j u) v")
                        .unsqueeze(3).to_broadcast([128, uB, 4, 2]), ACT.Copy)

                    # --- messages: [h*ex (64) | ex (4)] per gather region
                    msgA = mp.tile([128, uA, 68], BF16, tag="msgA")
                    nc.vector.tensor_tensor(
                        out=msgA[:, :, 0:64].rearrange("p t (h c e) -> p t h c e",
                                                       h=H, e=2),
                        in0=gA[:, :, 0:64].rearrange("p t (h c e) -> p t h c e", h=H, e=2),
                        in1=exLo2[:].unsqueeze(3).to_broadcast([128, uA, 4, 8, 2]),
                        op=ALU.mult)
                    nc.vector.tensor_copy(
                        out=msgA[:, :, 64:68],
                        in_=exLo[:].rearrange("p j u v -> p (j u) v"))
                    msgB = mp.tile([128, uB, 68], BF16, tag="msgB")
                    nc.vector.tensor_tensor(
                        out=msgB[:, :, 0:64].rearrange("p t (h c e) -> p t h c e",
                                                       h=H, e=2),
                        in0=gB[:, :, 0:64].rearrange("p t (h c e) -> p t h c e",
                                                     h=H, e=2),
                        in1=exHi2[:].unsqueeze(3).to_broadcast([128, uB, 4, 8, 2]),
                        op=ALU.mult)
                    nc.vector.tensor_copy(
                        out=msgB[:, :, 64:68],
                        in_=exHi[:].rearrange("p j u v -> p (j u) v"))

                    # --- aggregate per slot: 68-wide PSUM, 4 row blocks
                    U = pp.tile([128, 68], F32, tag="U")
                    for j in range(4):
                        for tt in range(nA):
                            nc.tensor.matmul(U[32 * j:32 * (j + 1), :],
                                             Sv[:, tt, j, :], msgA[:, j * nA + tt, :],
                                             start=(tt == 0), stop=False,
                                             tile_position=(0, 32 * j))
                        for tt in range(nB0, tps):
                            nc.tensor.matmul(U[32 * j:32 * (j + 1), :],
                                             Sv[:, tt, j, :],
                                             msgB[:, j * nBn + tt - nB0, :],
                                             start=False, stop=(tt == tps - 1),
                                             tile_position=(0, 32 * j))

                    # --- epilogue: out = U/(den+eps) + bias, relu
                    Usb = ep.tile([128, 64], F32, tag="Usb")
                    nc.vector.tensor_copy(out=Usb[:], in_=U[:, 0:64])
                    rec = ep.tile([128, 4], F32, tag="rec")
                    nc.vector.tensor_scalar_add(rec[:], U[:, 64:68], EPS)
                    nc.vector.reciprocal(rec[:], rec[:])
                    outq = ep.tile([128, 64], F32, tag="outq")
                    nc.vector.tensor_tensor(
                        out=outq[:].rearrange("p (h c) -> p h c", h=H),
                        in0=Usb[:].rearrange("p (h c) -> p h c", h=H),
                        in1=rec[:].unsqueeze(2).to_broadcast([128, H, HID]),
                        op=ALU.mult)
                    bias = b1bc if layer == 1 else b2bc
                    nc.vector.tensor_tensor(out=outq[:], in0=outq[:], in1=bias[:],
                                            op=ALU.add)
                    nc.vector.tensor_scalar_max(outq[:], outq[:], 0.0)

                    # transpose out_quad (both layers need it)
                    nc.tensor.transpose(tp[0:64, :], outq[:], Isb[:])
                    tpsb = ep.tile([64, 128], BF16, tag="tpsb")
                    nc.scalar.activation(tpsb[:], tp[0:64, :], ACT.Copy)

                    if layer == 1:
                        # layer-2 table rows + a_d2 for this quad's 128 nodes
                        nc.tensor.matmul(scr[:, 256:328], tpsb[:], W2sb[:],
                                         start=True, stop=True)
                        t2sb = ep.tile([128, 68], BF16, tag="t2sb")
                        nc.scalar.activation(t2sb[:], scr[:, 256:324], ACT.Copy)
                        nc.vector.tensor_copy(out=ad2sb[:, q, :], in_=scr[:, 324:328])
                        nc.sync.dma_start(
                            d_T2loc.ap()[128 * q:128 * (q + 1), :], t2sb[:])
                    else:
                        # head: logits into zall; exp-sum into smsb
                        nc.tensor.matmul(scr[:, 256:256 + OUT], tpsb[:], LWsb[:],
                                         start=True, stop=True)
                        nc.vector.tensor_tensor(out=zall[:, q, :],
                                                in0=scr[:, 256:256 + OUT],
                                                in1=lbbc[:], op=ALU.add)
                        mx = ep.tile([128, 1], F32, tag="mx")
                        nc.vector.reduce_max(mx[:], zall[:, q, :],
                                             axis=mybir.AxisListType.X)
                        nc.vector.tensor_scalar(out=zall[:, q, :], in0=zall[:, q, :],
                                                scalar1=mx[:], scalar2=None,
                                                op0=ALU.subtract)
                        ez = ep.tile([128, OUT], BF16, tag="ez")
                        nc.scalar.activation(ez[:], zall[:, q, :], ACT.Exp,
                                             accum_out=smsb[:, q:q + 1])

        edge_layer(1, d_T1, ad1sb)

        # one AllGather of the layer-2 table (device-major concat)
        nc.gpsimd.collective_compute(
            "AllGather", mybir.AluOpType.bypass,
            replica_groups=[list(range(C))],
            ins=[d_T2loc.ap()],
            outs=[d_T2t.ap()],
        )
        nc.sync.dma_start(d_T2.ap().bitcast(BF16)[:, 0:68], d_T2t.ap()[:, :])

        edge_layer(2, d_T2, ad2sb)

        # ---------------- tail: log_softmax finish + single output DMA -------
        with ExitStack() as ph:
            tpool = ph.enter_context(tc.tile_pool(name="tail", bufs=1))
            lnall = tpool.tile([128, qpd], F32)
            nc.scalar.activation(lnall[:], smsb[:], ACT.Ln)
            nc.vector.tensor_tensor(
                out=zall[:],
                in0=zall[:],
                in1=lnall[:].unsqueeze(2).to_broadcast([128, qpd, OUT]),
                op=ALU.subtract)
            nc.sync.dma_start(t_out[:, :],
                              zall[:].rearrange("p q d -> p (q d)"))

    return nc


# ----------------------------------------------------------------------------
# public entry
# ----------------------------------------------------------------------------

def _prepare(inputs):
    x = np.asarray(inputs["x"], np.float32)
    ei = np.asarray(inputs["edge_index"], np.int64)
    ea = np.asarray(inputs["edge_attr"], np.float32)
    n = x.shape[0]
    loop = np.arange(n, dtype=np.int64)
    src = np.concatenate([ei[0], loop])
    dst = np.concatenate([ei[1], loop])
    mean_attr = ea.mean(axis=0)

    plan = _build_plan(src, dst, n)
    W1aug, W2aug, Ve, LW, lb2p, b1, b2 = _fold_weights(
        np.asarray(inputs["W1"], np.float32), np.asarray(inputs["att_src1"], np.float32),
        np.asarray(inputs["att_dst1"], np.float32), np.asarray(inputs["We1"], np.float32),
        np.asarray(inputs["att_e1"], np.float32), np.asarray(inputs["b1"], np.float32),
        np.asarray(inputs["W2"], np.float32), np.asarray(inputs["att_src2"], np.float32),
        np.asarray(inputs["att_dst2"], np.float32), np.asarray(inputs["We2"], np.float32),
        np.asarray(inputs["att_e2"], np.float32), np.asarray(inputs["b2"], np.float32),
        np.asarray(inputs["lw1"], np.float32), np.asarray(inputs["lb1"], np.float32),
        np.asarray(inputs["lw2"], np.float32), np.asarray(inputs["lb2"], np.float32))
    per_core, xT, tq = _host_arrays(plan, x, src, dst, ea, mean_attr, n, Ve)

    bf16 = _bf16()
    cst = np.zeros((1, 256), np.float32)
    cst[0, 0:64] = b1
    cst[0, 64:128] = b2
    cst[0, 128:128 + OUT] = lb2p
    cst[0, 168:168 + NSLOT] = np.arange(NSLOT, dtype=np.float32)
    ident = np.eye(128, dtype=np.float32)
    bdmask = np.zeros((128, 16), np.float32)
    for j in range(4):
        bdmask[32 * j:32 * (j + 1), 4 * j:4 * (j + 1)] = 1.0
    bdmask = bdmask.astype(bf16)

    in_maps = []
    for d in range(C):
        pc = per_core[d]
        in_maps.append({
            "xT": xT, "xTloc": pc["xTloc"], "comb": pc["comb"],
            "W1aug": W1aug, "W2aug": W2aug, "LW": LW, "cst": cst,
            "ident": ident, "bdmask": bdmask,
        })
    return plan, in_maps, tq


def _assemble(plan, outs, n):
    bin_of, pos_of, spd, qpd = plan["bin_of"], plan["pos_of"], plan["spd"], plan["qpd"]
    dev = bin_of // spd
    s = bin_of % spd
    q = s // 4
    u = (s % 4) * NSLOT + pos_of
    stacked = np.stack([np.asarray(o, np.float32).reshape(128, qpd, OUT)
                        for o in outs])
    return stacked[dev[:n], u[:n], q[:n]]


def _run(inputs, trace=False, **spmd_kwargs):
    from concourse.bass_utils import run_bass_kernel_spmd

    plan, in_maps, tq = _prepare(inputs)
    nc = _build_nc(plan["ng"], plan["npd"], plan["spd"], plan["tps"], tq,
                   plan["nA"], plan["nB0"], plan["X"])
    nc.compile()
    res = run_bass_kernel_spmd(nc, in_maps, core_ids=list(range(C)), trace=trace,
                               **spmd_kwargs)
    outs = [r["out"] for r in res.results]
    return _assemble(plan, outs, inputs["x"].shape[0]), res


def kernel(**inputs):
    out, _ = _run(inputs)
    return out


# revision 35
# speedup vs baseline: 1.0008x; 1.0008x over previous
"""GATNet (2x GATConv + MLP head + log_softmax) on 8 Trainium2 NeuronCores.

Strategy (dst-partitioned message passing, v3):
  - Host assigns destination nodes to 8 devices x SPD slots (32 nodes/slot),
    balancing in-edge counts so every slot has exactly TPS 128-edge tiles.
    Every device runs an identical program; per-device data differ.
  - Layer tables T = [h(64) | a_s(4) | pad] are single 256B bf16 rows, one per
    node, [ng, 128].  int16 gather indices only reach 32767, so each slot's
    edges are sorted low-half sources first and every chunk runs TWO
    dma_gathers: A over tiles [0, nA) against table rows [0, ng/2), B over
    tiles [nB0, tps) against rows [ng/2, ng).  Only ~(nA+tps-nB0)/tps of the
    pair-gather bytes move, which is what the gather costs on the Pool queue.
  - Per chunk (4 slots = 128 dst nodes) attention is built edge-wise (bf16),
    low/high exp weights are region-masked, and messages aggregate into one
    68-wide PSUM accumulator at 4 partition offsets via one-hot matmuls.
  - Layer-2 table rows are produced inside layer 1's epilogue (each device
    computes rows only for its own nodes = device-major global numbering) and
    shared with a single AllGather between the layers.
  - a_e for both layers is computed once in layer 1 and parked in SBUF; a_d
    tables also stay resident in SBUF.  log_softmax's Ln runs once at the end
    (avoids per-chunk activation-table reloads); output leaves in one DMA.
"""

import numpy as np

# model constants (fixed by the problem)
IN = 128
HID = 16
OUT = 40
H = 4
ED = 16
HC = 64  # HID * H
NEG = 0.2
EPS = 1e-16

C = 8          # NeuronCores
NSLOT = 32     # nodes per slot (= one-hot width, PSUM col-block)


def _bf16():
    import concourse.mybir as mybir
    return mybir.dt.np(mybir.dt.bfloat16)


# ----------------------------------------------------------------------------
# host-side plan: balance nodes into (device, slot) bins, lay out edge shards
# ----------------------------------------------------------------------------

def _build_plan(src, dst, n_nodes):
    """Returns a dict with the full sharding plan. src/dst include self-loops."""
    import heapq

    deg = np.bincount(dst, minlength=n_nodes).astype(np.int64)
    e_tot = src.shape[0]

    def try_pack(nbins, cap_e, lo_deg=None, cap_lo=None, cap_hi=None):
        # LPT: heaviest nodes first into least-loaded feasible bin; optional
        # second dimension caps the low-half / high-half in-edge loads.
        order = np.argsort(-deg, kind="stable")
        loads = [(0, b) for b in range(nbins)]
        heapq.heapify(loads)
        bin_of_t = np.empty(n_nodes, np.int64)
        bin_cnt = np.zeros(nbins, np.int64)
        bin_load = np.zeros(nbins, np.int64)
        bin_lo = np.zeros(nbins, np.int64)
        for nd in order:
            d = int(deg[nd])
            lo = int(lo_deg[nd]) if lo_deg is not None else 0
            hi = d - lo
            spill = []
            placed = False
            while loads:
                l, b = heapq.heappop(loads)
                ok = bin_cnt[b] < NSLOT and bin_load[b] + d <= cap_e
                if ok and lo_deg is not None:
                    ok = (bin_lo[b] + lo <= cap_lo
                          and (bin_load[b] + d) - (bin_lo[b] + lo) <= cap_hi)
                if ok:
                    bin_of_t[nd] = b
                    bin_cnt[b] += 1
                    bin_load[b] += d
                    bin_lo[b] += lo
                    heapq.heappush(loads, (bin_load[b], b))
                    placed = True
                    break
                elif bin_cnt[b] < NSLOT:
                    spill.append((l, b))
                # full bins are dropped
            for it in spill:
                heapq.heappush(loads, it)
            if not placed:
                return None
        return bin_of_t

    # search (slots-per-device, even tiles-per-slot) minimizing total tiles
    spd_min = 4 * int(np.ceil(n_nodes / (C * NSLOT * 4)))  # node-capacity floor
    best = None  # (tq, spd, tps, bin_of)
    for spd_try in range(spd_min, spd_min + 65, 4):
        nbins = C * spd_try
        tps_lo = int(np.ceil(e_tot / nbins / 128.0))
        tps_lo += tps_lo % 2  # ch = 4*tps must be a multiple of 8
        for tps_try in (tps_lo, tps_lo + 2):
            if best is not None and spd_try * tps_try >= best[0]:
                continue
            got = try_pack(nbins, tps_try * 128)
            if got is not None:
                best = (spd_try * tps_try, spd_try, tps_try, got)
                break
        if best is not None and (spd_try + 4) * 2 >= best[0]:
            break
    assert best is not None, "balancer failed"
    _, spd, tps, bin_of = best

    nbins = C * spd
    npd = spd * NSLOT
    ng = C * npd

    def finish(bin_of):
        # position of each node within its bin; device-major global numbering
        pos_of = np.zeros(n_nodes, np.int64)
        fill = np.zeros(nbins, np.int64)
        for nd in range(n_nodes):
            b = bin_of[nd]
            pos_of[nd] = fill[b]
            fill[b] += 1
        node2g = ((bin_of // spd) * npd + (bin_of % spd) * NSLOT + pos_of
                  ).astype(np.int64)
        return pos_of, node2g

    pos_of, node2g = finish(bin_of)

    # table half boundary: low rows [0, X), high rows [X, ng).  Both gather
    # index spaces must fit int16.
    X = int(round(0.4435 * ng / 128.0)) * 128
    X = min(X, 32768)
    assert ng - X <= 32768

    # re-pack with per-bin low/high in-edge caps so the two gather spans are
    # as narrow as possible (lo <= 500 -> nA=4; hi <= 615 -> nB0>=3 for full
    # bins).  Low-degrees come from the first pack's numbering; the re-pack
    # perturbs halves only slightly, so re-measure and take actual spans.
    lo_src = (node2g[src] < X)
    lo_deg_nd = np.bincount(dst[lo_src], minlength=n_nodes).astype(np.int64)
    repack = None
    for cap_lo, cap_hi in ((500, 600), (500, 615), (511, 630)):
        repack = try_pack(nbins, tps * 128, lo_deg=lo_deg_nd,
                          cap_lo=cap_lo, cap_hi=cap_hi)
        if repack is not None:
            break
    if repack is not None:
        bin_of = repack
        pos_of, node2g = finish(bin_of)

    # edges per destination bin, LOW-half sources first, then high, then pad
    ebin = bin_of[dst]
    half = (node2g[src] >= X).astype(np.int64)
    order = np.lexsort((half, ebin))          # by bin, low-src first (stable)
    counts = np.bincount(ebin, minlength=nbins)
    lowcnt = np.bincount(ebin[half == 0], minlength=nbins)
    cap = tps * 128
    assert counts.max() <= cap
    starts = np.zeros(nbins + 1, np.int64)
    np.cumsum(counts, out=starts[1:])
    rank = np.arange(e_tot, dtype=np.int64) - starts[ebin[order]]
    canvas = np.full((nbins, cap), -1, np.int64)       # edge id or -1 pad
    canvas[ebin[order], rank] = order

    # static gather spans: A covers tiles [0, nA), B covers [nB0, tps)
    nA = int(np.ceil(lowcnt.max() / 128.0))
    nB0 = int(lowcnt.min() // 128)
    assert 0 <= nB0 <= nA <= tps

    return dict(
        spd=spd, tps=tps, npd=npd, ng=ng, nbins=nbins, qpd=spd // 4,
        nA=nA, nB0=nB0, X=X, bin_of=bin_of, pos_of=pos_of, node2g=node2g,
        canvas=canvas,
    )


def _host_arrays(plan, x, src, dst, edge_attr, mean_attr, n_nodes, Ve):
    """Per-core input arrays."""
    bf16 = _bf16()
    spd, tps, npd, ng = plan["spd"], plan["tps"], plan["npd"], plan["ng"]
    nA, nB0, X = plan["nA"], plan["nB0"], plan["X"]
    nBn = tps - nB0
    node2g, pos_of, canvas = plan["node2g"], plan["pos_of"], plan["canvas"]
    tq = spd * tps                       # 128-edge tiles per device
    ch = 4 * tps
    nq = tq // ch                        # chunks
    e0 = edge_attr.shape[0]

    def wrap16(a):  # [n] i16 -> [128, n//16] gather-index wrapping
        return np.tile(a.reshape(-1, 16).T, (8, 1))

    # permuted node features, transposed: xT [IN, ng] (bf16)
    xg = np.zeros((ng, IN), np.float32)
    xg[node2g] = np.asarray(x, np.float32)
    xT = np.ascontiguousarray(xg.T.astype(bf16))

    per_core = []
    for d in range(C):
        cv = canvas[d * spd:(d + 1) * spd].reshape(tq, 128)  # [tile, lane]
        valid = cv >= 0
        eid = np.where(valid, cv, 0)
        srcg = np.where(valid, node2g[src[eid]], 0)          # [tq, 128]
        hi = (srcg >= X)
        idxA = np.where(hi, 0, srcg).astype(np.int16)
        idxB = np.where(hi, srcg - X, 0).astype(np.int16)
        m4 = np.repeat(hi.astype(bf16).T[:, :, None], 4, axis=2
                       ).reshape(128, tq * 4).view(np.int16)  # [128, tq*4]
        drel = np.where(valid, pos_of[dst[eid]], -1)          # [tq, 128]
        # host-built one-hot S: [p, chunk, (b j s)] (bf16 bits)
        S_all = (drel[:, :, None] == np.arange(NSLOT)).astype(bf16)
        Sfull = (S_all.reshape(nq, 4, tps, 128, NSLOT)
                 .transpose(3, 0, 2, 1, 4)
                 .reshape(128, nq, tps * 4 * NSLOT)).view(np.int16)

        # host-computed a_e for both layers: [128, tq, 8] bf16 bits
        ea = np.zeros((tq, 128, ED), np.float32)
        sel = valid & (eid < e0)
        ea[sel] = edge_attr[eid[sel]]
        loop_sel = valid & (eid >= e0)
        ea[loop_sel] = mean_attr
        ae8 = (ea.reshape(-1, ED) @ Ve).reshape(tq, 128, 8)
        ae8 = np.ascontiguousarray(ae8.transpose(1, 0, 2)).astype(bf16)

        # per-chunk comb block: [idxA | idxB | m4 | ae1 | ae2 | S] int16
        cwA, cwB = 32 * nA, 32 * nBn
        cw = cwA + cwB + 12 * ch + NSLOT * ch
        comb = np.empty((128, nq * cw), np.int16)
        t4 = np.arange(tq).reshape(nq, 4, tps)                # chunk, j, tt
        ae1b = ae8[:, :, 0:4].reshape(128, tq * 4).view(np.int16)
        ae2b = ae8[:, :, 4:8].reshape(128, tq * 4).view(np.int16)
        for q in range(nq):
            blk = comb[:, q * cw:(q + 1) * cw]
            tA = t4[q, :, 0:nA].reshape(-1)                   # u = j*nA+tt
            tB = t4[q, :, nB0:tps].reshape(-1)
            o = 0
            blk[:, o:o + cwA] = wrap16(idxA[tA].reshape(-1)); o += cwA
            blk[:, o:o + cwB] = wrap16(idxB[tB].reshape(-1)); o += cwB
            blk[:, o:o + 4 * ch] = m4[:, q * 4 * ch:(q + 1) * 4 * ch]; o += 4 * ch
            blk[:, o:o + 4 * ch] = ae1b[:, q * 4 * ch:(q + 1) * 4 * ch]; o += 4 * ch
            blk[:, o:o + 4 * ch] = ae2b[:, q * 4 * ch:(q + 1) * 4 * ch]; o += 4 * ch
            blk[:, o:] = Sfull[:, q, :]
        per_core.append(dict(
            comb=np.ascontiguousarray(comb),
            xTloc=np.ascontiguousarray(xT[:, d * npd:(d + 1) * npd]),
        ))
    return per_core, xT, tq


def _fold_weights(W1, att_s1, att_d1, We1, att_e1, b1,
                  W2, att_s2, att_d2, We2, att_e2, b2,
                  lw1, lb1, lw2, lb2):
    bf16 = _bf16()

    def head_fold(att):  # [H, HID] -> [HC, H] block diag columns
        A = np.zeros((HC, H), np.float32)
        for h in range(H):
            A[h * HID:(h + 1) * HID, h] = att[h]
        return A

    W1aug = np.concatenate([W1, W1 @ head_fold(att_s1), W1 @ head_fold(att_d1)], 1)
    W2aug = np.concatenate([W2, W2 @ head_fold(att_s2), W2 @ head_fold(att_d2)], 1)
    Ve = np.zeros((ED, 8), np.float32)
    for h in range(H):
        Ve[:, h] = We1[:, h * HID:(h + 1) * HID] @ att_e1[h]
        Ve[:, 4 + h] = We2[:, h * HID:(h + 1) * HID] @ att_e2[h]
    LW = (lw1 @ lw2).astype(np.float32)
    lb2p = (lb1 @ lw2 + lb2).astype(np.float32)
    return (W1aug.astype(bf16), W2aug.astype(bf16), Ve,
            LW.astype(bf16), lb2p, b1.astype(np.float32), b2.astype(np.float32))


# ----------------------------------------------------------------------------
# the bass program (identical for all cores)
# ----------------------------------------------------------------------------

def _build_nc(ng, npd, spd, tps, tq, nA, nB0, X):
    import concourse.bass as bass
    import concourse.mybir as mybir
    import concourse.tile as tile
    from concourse import bacc
    from contextlib import ExitStack

    F32 = mybir.dt.float32
    BF16 = mybir.dt.bfloat16
    I16 = mybir.dt.int16
    ALU = mybir.AluOpType
    ACT = mybir.ActivationFunctionType

    ch = 4 * tps          # tiles per chunk (one quad = 4 slots)
    qpd = spd // 4        # chunks per device per layer
    nt = ng // 128        # node tiles (table build)
    jpd = npd // 128      # local 128-node groups (== qpd)
    ngr = ch // 8         # eaTg groups per chunk
    nBn = tps - nB0
    ov = nA - nB0         # mixed tiles per slot
    uA, uB = 4 * nA, 4 * nBn
    cwA, cwB = 32 * nA, 32 * nBn
    cw = cwA + cwB + 12 * ch + NSLOT * ch

    nc = bacc.Bacc(None, target_bir_lowering=False)

    # kernel IO
    t_xT = nc.dram_tensor("xT", [128, ng], BF16, kind="ExternalInput")
    t_xTl = nc.dram_tensor("xTloc", [128, npd], BF16, kind="ExternalInput")
    t_comb = nc.dram_tensor("comb", [128, (tq // ch) * cw], I16, kind="ExternalInput")
    t_W1 = nc.dram_tensor("W1aug", [128, 72], BF16, kind="ExternalInput")
    t_W2 = nc.dram_tensor("W2aug", [64, 72], BF16, kind="ExternalInput")
    t_LW = nc.dram_tensor("LW", [64, OUT], BF16, kind="ExternalInput")
    t_cst = nc.dram_tensor("cst", [1, 256], F32, kind="ExternalInput")
    # cst row: [b1(64) | b2(64) | lb2p(40) | iota32(32) | pad]
    t_mask = nc.dram_tensor("bdmask", [128, 16], BF16, kind="ExternalInput")
    t_I = nc.dram_tensor("ident", [128, 128], F32, kind="ExternalInput")
    t_out = nc.dram_tensor("out", [128, qpd * OUT], F32, kind="ExternalOutput")

    # node tables: tight 136B rows for builds/collective; padded 256B rows
    # (DRAM->DRAM expanded) for the 256B-granularity gathers
    I32 = mybir.dt.int32
    d_T1t = nc.dram_tensor("T1t", [ng, 68], BF16)
    d_T1 = nc.dram_tensor("T1", [ng, 64], I32)
    d_T2loc = nc.dram_tensor("T2loc", [npd, 68], BF16)
    d_T2t = nc.dram_tensor("T2t", [ng, 68], BF16, addr_space="Shared")
    d_T2 = nc.dram_tensor("T2", [ng, 64], I32)

    with tile.TileContext(nc) as tc, ExitStack() as top:
        cp = top.enter_context(tc.tile_pool(name="consts", bufs=1))
        pers = top.enter_context(tc.tile_pool(name="persist", bufs=1))

        W1sb = cp.tile([128, 72], BF16)
        W2sb = cp.tile([64, 72], BF16)
        LWsb = cp.tile([64, OUT], BF16)
        Isb = cp.tile([128, 128], F32)
        maskb = cp.tile([128, 16], BF16)
        b1bc = cp.tile([128, 64], F32)
        b2bc = cp.tile([128, 64], F32)
        lbbc = cp.tile([128, OUT], F32)
        Ib16 = cp.tile([128, 128], BF16)
        nc.sync.dma_start(W1sb[:], t_W1[:, :])
        nc.sync.dma_start(W2sb[:], t_W2[:, :])
        nc.sync.dma_start(LWsb[:], t_LW[:, :])
        nc.sync.dma_start(Isb[:], t_I[:, :])
        nc.sync.dma_start(maskb[:], t_mask[:, :])
        nc.sync.dma_start(b1bc[:], t_cst[:, 0:64].partition_broadcast(128))
        nc.sync.dma_start(b2bc[:], t_cst[:, 64:128].partition_broadcast(128))
        nc.sync.dma_start(lbbc[:], t_cst[:, 128:128 + OUT].partition_broadcast(128))
        nc.vector.tensor_copy(out=Ib16[:], in_=Isb[:])

        # persistent SBUF state
        ad1sb = pers.tile([128, jpd, 4], BF16)      # layer-1 a_d per local node
        ad2sb = pers.tile([128, jpd, 4], BF16)      # layer-2 a_d per local node
        zall = pers.tile([128, qpd, OUT], F32)      # head logits (shifted)
        smsb = pers.tile([128, qpd], F32)           # softmax sums

        # ---------------- phase A1: T1 = [x@W1 | a_s1]; local a_d1 ----------
        with ExitStack() as ph:
            ap = ph.enter_context(tc.tile_pool(name="pa_sb", bufs=3))
            app = ph.enter_context(tc.tile_pool(name="pa_ps", bufs=2, space="PSUM"))
            for it, i0 in enumerate(range(0, nt, 8)):
                bs = min(8, nt - i0)
                xt = ap.tile([128, 8 * 128], BF16, tag="xt")
                ldeng = nc.sync if it % 2 == 0 else nc.scalar
                ldeng.dma_start(xt[:, 0:128 * bs], t_xT[:, 128 * i0:128 * (i0 + bs)])
                ps0 = app.tile([128, 4, 72], F32, tag="ps0")
                ps1 = app.tile([128, 4, 72], F32, tag="ps1")
                for c in range(bs):
                    pst = ps0 if c < 4 else ps1
                    nc.tensor.matmul(pst[:, c % 4, :], xt[:, 128 * c:128 * (c + 1)],
                                     W1sb[:], start=True, stop=True)
                hsb = ap.tile([128, 8, 68], BF16, tag="hsb")
                nc.vector.tensor_copy(out=hsb[:, 0:4, :], in_=ps0[:, :, 0:68])
                if bs > 4:
                    nc.vector.tensor_copy(out=hsb[:, 4:bs, :],
                                          in_=ps1[:, 0:bs - 4, 0:68])
                weng = nc.gpsimd if it % 2 == 0 else nc.scalar
                weng.dma_start(
                    d_T1t.ap()[128 * i0:128 * (i0 + bs), :].rearrange(
                        "(c r) d -> r c d", c=bs),
                    hsb[:, 0:bs, :])
            for jj0 in range(0, jpd, 8):
                bs = min(8, jpd - jj0)
                xt = ap.tile([128, 8 * 128], BF16, tag="xt")
                nc.sync.dma_start(xt[:, 0:128 * bs], t_xTl[:, 128 * jj0:128 * (jj0 + bs)])
                psa = app.tile([128, 32], F32, tag="psa")
                for c in range(bs):
                    nc.tensor.matmul(psa[:, 4 * c:4 * (c + 1)],
                                     xt[:, 128 * c:128 * (c + 1)],
                                     W1sb[:, 68:72], start=True, stop=True)
                nc.vector.tensor_copy(
                    out=ad1sb[:, jj0:jj0 + bs, :],
                    in_=psa[:, 0:4 * bs].rearrange("p (c v) -> p c v", v=4))
            # expand tight rows into the 256B-granularity gather table
            nc.sync.dma_start(d_T1.ap().bitcast(BF16)[:, 0:68], d_T1t.ap()[:, :])

        # ---------------- edge phase (shared for both layers) ----------------
        def edge_layer(layer, tbl, adsb):
            with ExitStack() as ph:
                ip = ph.enter_context(tc.tile_pool(name=f"l{layer}_idx", bufs=4))
                gp = ph.enter_context(tc.tile_pool(name=f"l{layer}_g", bufs=4))
                sp = ph.enter_context(tc.tile_pool(name=f"l{layer}_s", bufs=3))
                mp = ph.enter_context(tc.tile_pool(name=f"l{layer}_m", bufs=3))
                ep = ph.enter_context(tc.tile_pool(name=f"l{layer}_e", bufs=3))
                pp = ph.enter_context(tc.tile_pool(name=f"l{layer}_ps", bufs=2, space="PSUM"))
                p1 = ph.enter_context(tc.tile_pool(name=f"l{layer}_p1", bufs=2, space="PSUM"))

                tlo = tbl.ap()[0:X, :]
                thi = tbl.ap()[X:ng, :]

                for q in range(qpd):
                    c0 = ch * q
                    comb = ip.tile([128, cw], I16, tag="comb")
                    nc.sync.dma_start(comb[:], t_comb[:, cw * q:cw * (q + 1)])
                    idxAv = comb[:, 0:cwA]
                    idxBv = comb[:, cwA:cwA + cwB]
                    m4 = comb[:, cwA + cwB:cwA + cwB + 4 * ch].bitcast(BF16)
                    m4v = m4.rearrange("p (j b v) -> p j b v", j=4, v=4)
                    aeo = cwA + cwB + 4 * ch + (0 if layer == 1 else 4 * ch)
                    aev = (comb[:, aeo:aeo + 4 * ch].bitcast(BF16)
                           .rearrange("p (t v) -> p t v", v=4))
                    Sv = (comb[:, cwA + cwB + 12 * ch:cw].bitcast(BF16)
                          .rearrange("p (b j s) -> p b j s", j=4, s=NSLOT))

                    gA8 = gp.tile([128, uA, 64], I32, tag="gA")
                    nc.gpsimd.dma_gather(
                        out_ap=gA8[:], in_ap=tlo, idxs_ap=idxAv,
                        num_idxs=uA * 128, num_idxs_reg=uA * 128, elem_size=64,
                        single_packet=False)
                    gB8 = gp.tile([128, uB, 64], I32, tag="gB")
                    nc.gpsimd.dma_gather(
                        out_ap=gB8[:], in_ap=thi, idxs_ap=idxBv,
                        num_idxs=uB * 128, num_idxs_reg=uB * 128, elem_size=64,
                        single_packet=False)
                    gA = gA8[:].bitcast(BF16)
                    gB = gB8[:].bitcast(BF16)
                    gAv = gA.rearrange("p (j u) d -> p j u d", j=4)
                    gBv = gB.rearrange("p (j u) d -> p j u d", j=4)

                    # --- a_d expansion: S^T via PE, block-diag a_d matmul
                    bd = ip.tile([128, 16], BF16, tag="bd")
                    nc.vector.tensor_tensor(
                        out=bd[:],
                        in0=adsb[:, q, :].unsqueeze(1).to_broadcast([128, 4, 4]),
                        in1=maskb[:].rearrange("p (j v) -> p j v", v=4),
                        op=ALU.mult)
                    stp = p1.tile([128, tps * 128], BF16, tag="stp")
                    for b in range(tps):
                        nc.tensor.transpose(
                            stp[:, 128 * b:128 * (b + 1)],
                            Sv[:, b, :, :].rearrange("p a w -> p (a w)"), Ib16[:])
                    sts = sp.tile([128, tps * 128], BF16, tag="sts")
                    nc.scalar.activation(sts[:], stp[:], ACT.Copy)
                    scr = p1.tile([128, 328], F32, tag="scr")
                    alad = scr[:, 0:tps * 16]
                    for b in range(tps):
                        nc.tensor.matmul(alad[:, 16 * b:16 * (b + 1)],
                                         sts[:, 128 * b:128 * (b + 1)],
                                         bd[:], start=True, stop=True)
                    aladb = ep.tile([128, tps * 16], BF16, tag="aladb")
                    nc.scalar.activation(aladb[:], alad[:], ACT.Copy)

                    # --- alpha = a_s[src](half-sel) + a_d[dst] + a_e
                    al = mp.tile([128, ch, 4], BF16, tag="al")
                    tp = scr[:, 128:256]
                    alv = al[:].rearrange("p (j b) v -> p j b v", j=4)
                    if nB0 > 0:
                        nc.vector.tensor_copy(out=alv[:, :, 0:nB0, :],
                                              in_=gAv[:, :, 0:nB0, 64:68])
                    if nA < tps:
                        nc.vector.tensor_copy(out=alv[:, :, nA:tps, :],
                                              in_=gBv[:, :, ov:nBn, 64:68])
                    if ov > 0:
                        tmp = mp.tile([128, 4, ov, 4], BF16, tag="tmp")
                        nc.vector.tensor_tensor(
                            out=tmp[:], in0=gBv[:, :, 0:ov, 64:68],
                            in1=gAv[:, :, nB0:nA, 64:68], op=ALU.subtract)
                        nc.vector.tensor_tensor(
                            out=tmp[:], in0=tmp[:],
                            in1=m4v[:, :, nB0:nA, :], op=ALU.mult)
                        nc.vector.tensor_tensor(
                            out=alv[:, :, nB0:nA, :], in0=gAv[:, :, nB0:nA, 64:68],
                            in1=tmp[:], op=ALU.add)
                    nc.vector.tensor_tensor(out=al[:], in0=al[:], in1=aev,
                                            op=ALU.add)
                    # += a_d (tile (j, b) lives at aladb[:, 16b + 4j : +4])
                    nc.vector.tensor_tensor(
                        out=al[:], in0=al[:],
                        in1=aladb[:].rearrange("p (b j v) -> p j b v", j=4, v=4),
                        op=ALU.add)
                    # leaky relu + exp (bf16)
                    lk = mp.tile([128, ch, 4], BF16, tag="lk")
                    nc.vector.tensor_scalar_mul(lk[:], al[:], NEG)
                    nc.vector.tensor_tensor(out=lk[:], in0=al[:], in1=lk[:], op=ALU.max)
                    exb = mp.tile([128, ch, 4], BF16, tag="exb")
                    nc.scalar.activation(exb[:], lk[:], ACT.Exp)
                    exbv = exb[:].rearrange("p (j b) v -> p j b v", j=4)

                    # region-masked exp weights: exLo (A tiles), exHi (B tiles)
                    exHi = mp.tile([128, 4, nBn, 4], BF16, tag="exHi")
                    if nA < tps:
                        nc.vector.tensor_copy(out=exHi[:, :, ov:nBn, :],
                                              in_=exbv[:, :, nA:tps, :])
                    if ov > 0:
                        nc.vector.tensor_tensor(
                            out=exHi[:, :, 0:ov, :], in0=exbv[:, :, nB0:nA, :],
                            in1=m4v[:, :, nB0:nA, :], op=ALU.mult)
                    exLo = mp.tile([128, 4, nA, 4], BF16, tag="exLo")
                    if nB0 > 0:
                        nc.vector.tensor_copy(out=exLo[:, :, 0:nB0, :],
                                              in_=exbv[:, :, 0:nB0, :])
                    if ov > 0:
                        nc.vector.tensor_tensor(
                            out=exLo[:, :, nB0:nA, :], in0=exbv[:, :, nB0:nA, :],
                            in1=exHi[:, :, 0:ov, :], op=ALU.subtract)
                    exLo2 = mp.tile([128, uA, 4, 2], BF16, tag="exLo2")
                    nc.scalar.activation(
                        exLo2[:],
                        exLo[:].rearrange("p j u v -> p (j u) v")
                        .unsqueeze(3).to_broadcast([128, uA, 4, 2]), ACT.Copy)
                    exHi2 = mp.tile([128, uB, 4, 2], BF16, tag="exHi2")
                    nc.scalar.activation(
                        exHi2[:],
                        exHi[:].rearrange("p j u v -> p (j u) v")
                        .unsqueeze(3).to_broadcast([128, uB, 4, 2]), ACT.Copy)

                    # --- messages: [h*ex (64) | ex (4)] per gather region
                    msgA = mp.tile([128, uA, 68], BF16, tag="msgA")
                    nc.vector.tensor_tensor(
                        out=msgA[:, :, 0:64].rearrange("p t (h c e) -> p t h c e",
                                                       h=H, e=2),
                        in0=gA[:, :, 0:64].rearrange("p t (h c e) -> p t h c e", h=H, e=2),
                        in1=exLo2[:].unsqueeze(3).to_broadcast([128, uA, 4, 8, 2]),
                        op=ALU.mult)
                    nc.vector.tensor_copy(
                        out=msgA[:, :, 64:68],
                        in_=exLo[:].rearrange("p j u v -> p (j u) v"))
                    msgB = mp.tile([128, uB, 68], BF16, tag="msgB")
                    nc.vector.tensor_tensor(
                        out=msgB[:, :, 0:64].rearrange("p t (h c e) -> p t h c e",
                                                       h=H, e=2),
                        in0=gB[:, :, 0:64].rearrange("p t (h c e) -> p t h c e",
                                                     h=H, e=2),
                        in1=exHi2[:].unsqueeze(3).to_broadcast([128, uB, 4, 8, 2]),
                        op=ALU.mult)
                    nc.vector.tensor_copy(
                        out=msgB[:, :, 64:68],
                        in_=exHi[:].rearrange("p j u v -> p (j u) v"))

                    # --- aggregate per slot: 68-wide PSUM, 4 row blocks
                    U = pp.tile([128, 68], F32, tag="U")
                    for j in range(4):
                        for tt in range(nA):
                            nc.tensor.matmul(U[32 * j:32 * (j + 1), :],
                                             Sv[:, tt, j, :], msgA[:, j * nA + tt, :],
                                             start=(tt == 0), stop=False,
                                             tile_position=(0, 32 * j))
                        for tt in range(nB0, tps):
                            nc.tensor.matmul(U[32 * j:32 * (j + 1), :],
                                             Sv[:, tt, j, :],
                                             msgB[:, j * nBn + tt - nB0, :],
                                             start=False, stop=(tt == tps - 1),
                                             tile_position=(0, 32 * j))

                    # --- epilogue: out = U/(den+eps) + bias, relu
                    Usb = ep.tile([128, 64], F32, tag="Usb")
                    nc.vector.tensor_copy(out=Usb[:], in_=U[:, 0:64])
                    rec = ep.tile([128, 4], F32, tag="rec")
                    nc.vector.tensor_scalar_add(rec[:], U[:, 64:68], EPS)
                    nc.vector.reciprocal(rec[:], rec[:])
                    outq = ep.tile([128, 64], F32, tag="outq")
                    nc.vector.tensor_tensor(
                        out=outq[:].rearrange("p (h c) -> p h c", h=H),
                        in0=Usb[:].rearrange("p (h c) -> p h c", h=H),
                        in1=rec[:].unsqueeze(2).to_broadcast([128, H, HID]),
                        op=ALU.mult)
                    bias = b1bc if layer == 1 else b2bc
                    nc.vector.tensor_tensor(out=outq[:], in0=outq[:], in1=bias[:],
                                            op=ALU.add)
                    nc.vector.tensor_scalar_max(outq[:], outq[:], 0.0)

                    # transpose out_quad (both layers need it)
                    nc.tensor.transpose(tp[0:64, :], outq[:], Isb[:])
                    tpsb = ep.tile([64, 128], BF16, tag="tpsb")
                    nc.scalar.activation(tpsb[:], tp[0:64, :], ACT.Copy)

                    if layer == 1:
                        # layer-2 table rows + a_d2 for this quad's 128 nodes
                        nc.tensor.matmul(scr[:, 256:328], tpsb[:], W2sb[:],
                                         start=True, stop=True)
                        t2sb = ep.tile([128, 68], BF16, tag="t2sb")
                        nc.scalar.activation(t2sb[:], scr[:, 256:324], ACT.Copy)
                        nc.vector.tensor_copy(out=ad2sb[:, q, :], in_=scr[:, 324:328])
                        nc.sync.dma_start(
                            d_T2loc.ap()[128 * q:128 * (q + 1), :], t2sb[:])
                    else:
                        # head: logits into zall; exp-sum into smsb
                        nc.tensor.matmul(scr[:, 256:256 + OUT], tpsb[:], LWsb[:],
                                         start=True, stop=True)
                        nc.vector.tensor_tensor(out=zall[:, q, :],
                                                in0=scr[:, 256:256 + OUT],
                                                in1=lbbc[:], op=ALU.add)
                        mx = ep.tile([128, 1], F32, tag="mx")
                        nc.vector.reduce_max(mx[:], zall[:, q, :],
                                             axis=mybir.AxisListType.X)
                        nc.vector.tensor_scalar(out=zall[:, q, :], in0=zall[:, q, :],
                                                scalar1=mx[:], scalar2=None,
                                                op0=ALU.subtract)
                        ez = ep.tile([128, OUT], BF16, tag="ez")
                        nc.scalar.activation(ez[:], zall[:, q, :], ACT.Exp,
                                             accum_out=smsb[:, q:q + 1])

        edge_layer(1, d_T1, ad1sb)

        # one AllGather of the layer-2 table (device-major concat)
        nc.gpsimd.collective_compute(
            "AllGather", mybir.AluOpType.bypass,
            replica_groups=[list(range(C))],
            ins=[d_T2loc.ap()],
            outs=[d_T2t.ap()],
        )
        nc.sync.dma_start(d_T2.ap().bitcast(BF16)[:, 0:68], d_T2t.ap()[:, :])

        edge_layer(2, d_T2, ad2sb)

        # ---------------- tail: log_softmax finish + single output DMA -------
        with ExitStack() as ph:
            tpool = ph.enter_context(tc.tile_pool(name="tail", bufs=1))
            lnall = tpool.tile([128, qpd], F32)
            nc.scalar.activation(lnall[:], smsb[:], ACT.Ln)
            nc.vector.tensor_tensor(
                out=zall[:],
                in0=zall[:],
                in1=lnall[:].unsqueeze(2).to_broadcast([128, qpd, OUT]),
                op=ALU.subtract)
            nc.sync.dma_start(t_out[:, :],
                              zall[:].rearrange("p q d -> p (q d)"))

    return nc


# ----------------------------------------------------------------------------
# public entry
# ----------------------------------------------------------------------------

def _prepare(inputs):
    x = np.asarray(inputs["x"], np.float32)
    ei = np.asarray(inputs["edge_index"], np.int64)
    ea = np.asarray(inputs["edge_attr"], np.float32)
    n = x.shape[0]
    loop = np.arange(n, dtype=np.int64)
    src = np.concatenate([ei[0], loop])
    dst = np.concatenate([ei[1], loop])
    mean_attr = ea.mean(axis=0)

    plan = _build_plan(src, dst, n)
    W1aug, W2aug, Ve, LW, lb2p, b1, b2 = _fold_weights(
        np.asarray(inputs["W1"], np.float32), np.asarray(inputs["att_src1"], np.float32),
        np.asarray(inputs["att_dst1"], np.float32), np.asarray(inputs["We1"], np.float32),
        np.asarray(inputs["att_e1"], np.float32), np.asarray(inputs["b1"], np.float32),
        np.asarray(inputs["W2"], np.float32), np.asarray(inputs["att_src2"], np.float32),
        np.asarray(inputs["att_dst2"], np.float32), np.asarray(inputs["We2"], np.float32),
        np.asarray(inputs["att_e2"], np.float32), np.asarray(inputs["b2"], np.float32),
        np.asarray(inputs["lw1"], np.float32), np.asarray(inputs["lb1"], np.float32),
        np.asarray(inputs["lw2"], np.float32), np.asarray(inputs["lb2"], np.float32))
    per_core, xT, tq = _host_arrays(plan, x, src, dst, ea, mean_attr, n, Ve)

    bf16 = _bf16()
    cst = np.zeros((1, 256), np.float32)
    cst[0, 0:64] = b1
    cst[0, 64:128] = b2
    cst[0, 128:128 + OUT] = lb2p
    cst[0, 168:168 + NSLOT] = np.arange(NSLOT, dtype=np.float32)
    ident = np.eye(128, dtype=np.float32)
    bdmask = np.zeros((128, 16), np.float32)
    for j in range(4):
        bdmask[32 * j:32 * (j + 1), 4 * j:4 * (j + 1)] = 1.0
    bdmask = bdmask.astype(bf16)

    in_maps = []
    for d in range(C):
        pc = per_core[d]
        in_maps.append({
            "xT": xT, "xTloc": pc["xTloc"], "comb": pc["comb"],
            "W1aug": W1aug, "W2aug": W2aug, "LW": LW, "cst": cst,
            "ident": ident, "bdmask": bdmask,
        })
    return plan, in_maps, tq


def _assemble(plan, outs, n):
    bin_of, pos_of, spd, qpd = plan["bin_of"], plan["pos_of"], plan["spd"], plan["qpd"]
    dev = bin_of // spd
    s = bin_of % spd
    q = s // 4
    u = (s % 4) * NSLOT + pos_of
    stacked = np.stack([np.asarray(o, np.float32).reshape(128, qpd, OUT)
                        for o in outs])
    return stacked[dev[:n], u[:n], q[:n]]


def _run(inputs, trace=False, **spmd_kwargs):
    from concourse.bass_utils import run_bass_kernel_spmd

    plan, in_maps, tq = _prepare(inputs)
    nc = _build_nc(plan["ng"], plan["npd"], plan["spd"], plan["tps"], tq,
                   plan["nA"], plan["nB0"], plan["X"])
    nc.compile()
    res = run_bass_kernel_spmd(nc, in_maps, core_ids=list(range(C)), trace=trace,
                               **spmd_kwargs)
    outs = [r["out"] for r in res.results]
    return _assemble(plan, outs, inputs["x"].shape[0]), res


def kernel(**inputs):
    out, _ = _run(inputs)
    return out
